# revision 36
# baseline (speedup 1.0000x reference)
"""Trainium2 Bass kernel for nn_Pol2VecBinary (Taylor-time pol2vec binary loss).

reference:
  loss = -sum log(p_mat), p_mat = p if event else 1-p, p = sigmoid(y),
  y[b,n] = gr[n] + gc[ci[b]] - dist[b,n]
  dist   = || sum_o z_rows[o,n,:] t_b^o/o!  -  z_cols[ci[b],:] ||_2

Rewritten (e = events.T in {0,1}):
  loss = sum_{b,n} [ softplus(y) - e*y ]
       = sum ln(1 + e^y)  -  sum e*(gr+gc)  +  sum e*dist
         '--- device ---'   '--- host ----'    '-- device --'
  e^y = e^{gc - dist} * e^{gr}

dist^2 is a single PE contraction over K=640 (bf16 inputs, fp32 PSUM):
  k in [0,512):   L[k=(o,d), n] = zr_c[o,n,d]       R[k,b] = -2*T[o,b]*zc_c[b,d]
  k in [512,640): Gram block: sum_j G36[j,n]*TT36[j,b] + ||zc_c||^2
                  (bf16 hi/lo split keeps the quadratic terms exact to ~2^-17)
zr/zc are centered by 0.5 on the order-0 row so bf16 products stay small.

Sharding: 8 cores = 2 n-halves x 4 b-quarters, pure data parallel; the 8
scalar partial sums are combined on the host.

Device pipeline per core (b 512 = 4 partition-tiles of 128, n 1024):
  PE : 40 bf16 matmuls -> 2 PSUM tiles [128, 2048] = dist^2
  ACT: 2x Sqrt (PSUM->SBUF)                              [sqrt table]
  DVE: e*d (int8 x f32 -> bf16) ; reduce_sum -> acc_ed
  ACT: 4x Exp(-d + gc_t bias) -> u (bf16)                [nl_exp table]
  DVE: 4x u * e^gr (bf16) -> v
  ACT: 1x Ln(v + 1) with fused accum -> acc_sp
All per-tensor input DMAs are packed partition-major into one DMA each.
"""

import math

import ml_dtypes
import numpy as np

import concourse.bacc as bacc
import concourse.bass as bass
import concourse.hw_specs as hw_specs
import concourse.mybir as mybir
from concourse.tile import TileContext

ORDER = 7
O = ORDER + 1  # 8
N = 2048
B = 2048
DIM = 64
NSPLIT = 2
BSPLIT = 4
NCORES = NSPLIT * BSPLIT
NC_ = N // NSPLIT   # 1024 n per core
BC_ = B // BSPLIT   # 512 b per core
KCH = 5             # k-chunks of 128 (4 main + 1 gram)
NBT = BC_ // 128    # 4 b-tiles per core
NPS = 2             # psum tiles per core, each [128, 2*NC_] covering 2 b-tiles
FW = NBT * NC_      # elementwise free width (4096)

F32 = mybir.dt.float32
BF16 = mybir.dt.bfloat16
I8 = mybir.dt.int8
F16 = mybir.dt.float16
AF = mybir.ActivationFunctionType
ALU = mybir.AluOpType

_CACHE = {}


def _patch_act_tables():
    """Force every activation onto the combined Exp+Ln table (one load).

    Table ids are positional indices into the compiler's act_info.json, so
    the dict ORDER must not change; instead empty out the other tables'
    function sets so the load-inserter can only choose the preferred one.
    """
    orig = hw_specs.get_activation_tables
    pref = "natural_log_exp_and_others"

    def tables(arch):
        t = dict(orig(arch))
        if pref in t:
            t = {k: (v if k == pref else set()) for k, v in t.items()}
        return t

    bacc.get_activation_tables = tables


def build_nc():
    _patch_act_tables()
    nc = bacc.Bacc()
    L_d = nc.dram_tensor("L", [128, KCH * NC_], BF16, kind="ExternalInput")
    R_d = nc.dram_tensor("R", [128, KCH * BC_], BF16, kind="ExternalInput")
    EV_d = nc.dram_tensor("EV", [128, FW], I8, kind="ExternalInput")
    EGR_d = nc.dram_tensor("EGR", [NC_], BF16, kind="ExternalInput")
    GC_d = nc.dram_tensor("GC", [128, NBT], F32, kind="ExternalInput")
    OUT_d = nc.dram_tensor("OUT", [128, 2], F32, kind="ExternalOutput")

    with TileContext(nc) as tc:
        with (
            tc.tile_pool(name="consts", bufs=1) as consts,
            tc.tile_pool(name="psum", bufs=NBT, space="PSUM") as psum,
        ):
            # ---- loads (per-k chunks so PE can start after the first) ----
            L_sb = consts.tile([128, KCH * NC_], BF16, tag="L")
            R_sb = consts.tile([128, KCH * BC_], BF16, tag="R")
            for k0, k1 in ((0, 2), (2, 4), (4, 5)):
                nc.scalar.dma_start(
                    out=R_sb[:, k0 * BC_:k1 * BC_],
                    in_=R_d[:, k0 * BC_:k1 * BC_])
                nc.sync.dma_start(
                    out=L_sb[:, k0 * NC_:k1 * NC_],
                    in_=L_d[:, k0 * NC_:k1 * NC_])
            ev = consts.tile([128, FW], I8, tag="EV")
            nc.sync.dma_start(out=ev, in_=EV_d[:])
            gc_sb = consts.tile([128, NBT], F32, tag="GC")
            nc.sync.dma_start(out=gc_sb, in_=GC_d[:])
            egr = consts.tile([128, NC_], BF16, tag="EGR")
            egr_ap = EGR_d[:]
            egr_bcast = bass.AP(
                tensor=egr_ap.tensor, offset=egr_ap.offset,
                ap=[[0, 128]] + [list(x) for x in egr_ap.ap],
            )
            nc.sync.dma_start(out=egr, in_=egr_bcast)

            acc = consts.tile([128, 2], F32, tag="ACC")
            nc.vector.memset(acc, 0.0)
            w_all = consts.tile([128, FW], F32, tag="W")
            d_all = consts.tile([128, FW], F32, tag="D")
            u_all = consts.tile([128, FW], BF16, tag="U")
            v_all = consts.tile([128, FW], BF16, tag="V")
            junk = consts.tile([128, FW], BF16, tag="JUNK")
            tr0 = consts.tile([128, FW], F16, tag="TR0")
            tr1 = consts.tile([128, FW // 2], F16, tag="TR1")
            tr2 = consts.tile([128, FW // 4], F16, tag="TR2")
            tr3 = consts.tile([128, FW // 8], F16, tag="TR3")
            lnout = consts.tile([128, FW // 8], F16, tag="LNOUT")

            ones_t = consts.tile([128, 1], BF16, tag="ONES")
            nc.vector.memset(ones_t, 1.0)

            # ---- PE warm-up: dummy matmuls while the L/R DMAs stream in,
            # so the HAM clock-gate is released before the real work.
            # They scribble on psum tile 0, which the first real
            # accumulation group overwrites with start=True. ----
            warm_sb = consts.tile([128, 512], BF16, tag="WRM")
            nc.vector.memset(warm_sb, 0.0)
            ps_tiles = []
            for t in range(NBT):
                p_tile = psum.tile([128, NC_], F32, tag="P")
                ps_tiles.append(p_tile)
            for i in range(10):
                nc.tensor.matmul(ps_tiles[0][0:1, 0:512], lhsT=ones_t,
                                 rhs=warm_sb, start=True, stop=True)

            # ---- PE: t-outer; tile 0 is paced by the arriving L chunks,
            # tiles 1-3 then run at full speed with everything resident ----
            for t in range(NBT):
                for k in range(KCH):
                    for h in range(NC_ // 512):
                        nc.tensor.matmul(
                            ps_tiles[t][:, h * 512:(h + 1) * 512],
                            lhsT=R_sb[:, k * BC_ + t * 128:
                                      k * BC_ + (t + 1) * 128],
                            rhs=L_sb[:, k * NC_ + h * 512:
                                     k * NC_ + (h + 1) * 512],
                            start=(k == 0),
                            stop=(k == KCH - 1),
                        )

            # ---- per-b-tile pipeline (t-outer: psum t completes early
            # and the elementwise chain runs behind the PE) ----
            # d = sqrt(D2) computed as Exp(0.5*Ln(D2)) so the whole kernel
            # uses the single Exp+Ln act table (no table switching).
            for t in range(NBT):
                sl = slice(t * NC_, (t + 1) * NC_)
                P = ps_tiles[t]
                # w = ln(dist^2); d = exp(0.5 w) = dist; u = exp(gc - d)
                nc.scalar.activation(out=w_all[:, sl], in_=P, func=AF.Ln)
                nc.scalar.activation(out=d_all[:, sl], in_=w_all[:, sl],
                                     func=AF.Exp, scale=0.5)
                nc.scalar.activation(out=u_all[:, sl], in_=d_all[:, sl],
                                     func=AF.Exp, scale=-1.0,
                                     bias=gc_sb[:, t:t + 1])
                # e * d  and  v = u * e^gr
                nc.vector.tensor_mul(junk[:, sl], ev[:, sl], d_all[:, sl])
                nc.vector.tensor_mul(v_all[:, sl], u_all[:, sl], egr)
                # product tree, per-tile part: w0 = 1+v ; m1 = pair products
                nc.vector.tensor_scalar_add(tr0[:, sl], v_all[:, sl], 1.0)
                nc.vector.tensor_mul(
                    tr1[:, t * (NC_ // 2):(t + 1) * (NC_ // 2)],
                    tr0[:, t * NC_:t * NC_ + NC_ // 2],
                    tr0[:, t * NC_ + NC_ // 2:(t + 1) * NC_])
                # second tree level as soon as a tile PAIR is done
                if t % 2 == 1:
                    q = t // 2
                    nc.vector.tensor_mul(
                        tr2[:, q * (NC_ // 2):(q + 1) * (NC_ // 2)],
                        tr1[:, (t - 1) * (NC_ // 2):t * (NC_ // 2)],
                        tr1[:, t * (NC_ // 2):(t + 1) * (NC_ // 2)])

            # ---- acc_ed = sum e*d : partition-reduce on PE (ones matmul),
            # then a tiny free-dim reduce of the [1, 512] psum row.
            # (5th tag-P allocation: recycles psum slot 0 after its release)
            red_p = psum.tile([128, NC_], F32, tag="P")
            for c in range(FW // 512):
                nc.tensor.matmul(
                    red_p[0:1, 0:512], lhsT=ones_t,
                    rhs=junk[:, c * 512:(c + 1) * 512],
                    start=(c == 0), stop=(c == FW // 512 - 1),
                )
            nc.vector.reduce_sum(acc[0:1, 1:2], red_p[0:1, 0:512],
                                 axis=mybir.AxisListType.X)

            # ---- acc_sp = sum ln(1+v) = ln prod(1+v): finish the tree ----
            H2 = FW // 4   # tr2 is [128, 1024] live (built per tile-pair)
            nc.vector.tensor_mul(tr3, tr2[:, :H2 // 2], tr2[:, H2 // 2:])
            nc.scalar.activation(
                out=lnout, in_=tr3,
                func=AF.Ln, scale=1.0, bias=0.0, accum_out=acc[:, 0:1],
            )

            nc.sync.dma_start(out=OUT_d[:], in_=acc)
    nc.compile()
    return nc


def _to_bf16(x):
    return np.asarray(x, dtype=np.float32).astype(ml_dtypes.bfloat16)


def _pack(x, width):
    """[KCH*128, width] -> [128, KCH*width] partition-major."""
    k = x.shape[0] // 128
    return np.ascontiguousarray(
        x.reshape(k, 128, width).transpose(1, 0, 2).reshape(128, k * width))


def prepare_in_maps(events, col_indices, col_times, z_rows, z_cols,
                    gamma_rows, gamma_cols):
    """Host-side shard prep: per-core L/R/EV/EGR/GC + the host constant."""
    events = np.asarray(events)
    ci = np.asarray(col_indices)
    ct = np.asarray(col_times, dtype=np.float64)
    zr = np.asarray(z_rows, dtype=np.float32)
    zc = np.asarray(z_cols, dtype=np.float32)
    gr = np.asarray(gamma_rows, dtype=np.float32)
    gc = np.asarray(gamma_cols, dtype=np.float32)

    inv_fact = np.array([1.0 / math.factorial(o) for o in range(O)])
    T = ((ct[None, :] ** np.arange(O)[:, None]) * inv_fact[:, None]).astype(
        np.float32)  # [O, B]

    zr_c = zr.copy()
    zr_c[0] -= 0.5
    zc_sel = zc[ci] - 0.5          # [B, D]
    gc_sel = gc[ci]                # [B]

    # host-side part of the e*y term
    rs = events.sum(axis=1, dtype=np.int64)   # [N]
    cs = events.sum(axis=0, dtype=np.int64)   # [B]
    host_c = float(rs @ gr.astype(np.float64) + cs @ gc_sel.astype(np.float64))

    # main block [512, *]
    Lmain = np.transpose(zr_c, (0, 2, 1)).reshape(O * DIM, N)        # [512, N]
    Rmain = (-2.0 * T[:, None, :] * zc_sel.T[None, :, :]).reshape(
        O * DIM, B).astype(np.float32)                               # [512, B]

    # gram block [128, *]
    pairs = [(o, p) for o in range(O) for p in range(o, O)]          # 36
    Gf = np.einsum("ond,pnd->opn", zr_c, zr_c, optimize=True)        # [O,O,N]
    G36 = np.stack([(2.0 - (o == p)) * Gf[o, p] for (o, p) in pairs]
                   ).astype(np.float32)                              # [36, N]
    TT36 = np.stack([T[o] * T[p] for (o, p) in pairs])               # [36, B]
    cn = np.sum(zc_sel * zc_sel, axis=1, dtype=np.float64).astype(
        np.float32)                                                  # [B]

    def hi_lo(x):
        hi = _to_bf16(x).astype(np.float32)
        return hi, x - hi

    G_hi, G_lo = hi_lo(G36)
    TT_hi, TT_lo = hi_lo(TT36)
    cn_hi, cn_lo = hi_lo(cn)
    ones_n = np.ones((1, N), np.float32)

    Lg = np.concatenate([G_hi, G_hi, G_lo, ones_n, ones_n,
                         np.zeros((18, N), np.float32)])              # [128,N]
    Rg = np.concatenate([TT_hi, TT_lo, TT_hi, cn_hi[None], cn_lo[None],
                         np.zeros((18, B), np.float32)])              # [128,B]

    Lfull = _to_bf16(np.concatenate([Lmain, Lg]))   # [640, N] bf16
    Rfull = _to_bf16(np.concatenate([Rmain, Rg]))   # [640, B] bf16
    egr_full = _to_bf16(np.exp(gr.astype(np.float64)))  # [N] bf16

    in_maps = []
    for c in range(NCORES):
        ni, bi = divmod(c, BSPLIT)
        n0 = ni * NC_
        b0 = bi * BC_
        Lc = _pack(Lfull[:, n0:n0 + NC_], NC_)
        Rc = _pack(Rfull[:, b0:b0 + BC_], BC_)
        evc = _pack(events[n0:n0 + NC_, b0:b0 + BC_].T.astype(np.int8), NC_)
        egrc = np.ascontiguousarray(egr_full[n0:n0 + NC_])
        gcc = np.ascontiguousarray(gc_sel[b0:b0 + BC_].reshape(NBT, 128).T)
        in_maps.append({"L": Lc, "R": Rc, "EV": evc, "EGR": egrc, "GC": gcc})
    return in_maps, host_c


def finalize(results, host_c):
    """loss = sum ln(1+e^y)  +  sum e*dist  -  sum e*(gr+gc)."""
    total = -host_c
    for r in results:
        out = np.asarray(r["OUT"], dtype=np.float64)  # [128, 2]
        total += out[:, 0].sum() + out[0, 1]
    return np.asarray(total, dtype=np.float32)


def kernel(**inputs):
    from concourse.bass_utils import run_bass_kernel_spmd

    if "nc" not in _CACHE:
        _CACHE["nc"] = build_nc()
    nc = _CACHE["nc"]
    in_maps, host_c = prepare_in_maps(**inputs)
    res = run_bass_kernel_spmd(nc, in_maps, list(range(NCORES)))
    return finalize(res.results, host_c)
